# revision 13
# baseline (speedup 1.0000x reference)
"""Trainium2 Bass kernel for nn_AttentionLayer (B=4, S=4096, D=128, fp32).

Sharding: batch (4) x query-half (2) across 8 NeuronCores. Each core computes
single-head attention for one batch element over a 2048-query slice with full
4096-key context.

Math (host-side algebra):
  scores[s,t] = q[s]@k[t]/sqrt(d) = x[s] G x[t]^T + alpha[t] + const(s),
  G = Wq^T Wk / sqrt(d), alpha[t] = x[t]@(Wk^T bq)/sqrt(d).
  const(s) cancels in softmax; bk drops entirely; bv is added at the end.
  The Q projection disappears: k''[t] = x[t] G^T is the only "key" tensor and
  raw x columns are the scores moving operand. Key order is free (softmax
  sums over keys), so the host rolls x^T per core to put the core's query
  slice at columns 0:SQ - no separate query buffer or DMA.

Per-core dataflow (mode-dependent):
  k''^T = gT.T @ x^T                  (PE, bf16, N=512 x8)
  [V | alpha] chunks = x^T-chunk.T @ wv_aug   (PE, bf16, N=130 x32)
  scoresT[t-chunk, s] = k''-chunk.T @ x^T[:, queries]   (PE bf16 -> PSUM f32)
  et = exp(scores + alpha - SHIFT)    (ACT, PSUM -> SBUF)
  mode "bf16": et bf16; AV bf16 matmuls; denominator chunk-accumulated on DVE
    (bf16 2x) folded by one ones-matmul per pass.
  mode "fp8": et fp8e4 written into pair tiles [128, 2*sw]; AV and denominator
    are fp8 DoubleRow matmuls (2 key-chunks per matmul); denominator costs no
    DVE time. Optionally (noff>0) some chunks' exp runs on DVE via a
    Schraudolph bit-trick straight to fp8 bits (one tensor_scalar per chunk),
    offloading the ACT engine.
  out^T = AV / denom + bv  (DVE), DMA'd out as [e, s]; host flips layout.

A dependency-free warm-up burst of matmuls on zeroed tiles runs while the
input DMAs land: the PE's HAM clock-gate needs ~3.4us of sustained busy to
lift the 1.2GHz cold throttle to 2.4GHz; without the burst the projection
phase runs cold and the gate only lifts ~25us into the kernel.
"""

import sys

import numpy as np

for _p in ("/opt/trn_rl_repo", "/opt/pypackages"):
    if _p not in sys.path:
        sys.path.append(_p)

import ml_dtypes

B, S, D = 4, 4096, 128
N_CORES = 8
SQ = S // 2  # queries per core
SHIFT = 3.5  # subtracted inside exp; cancels in softmax, keeps et fp8-safe

# Schraudolph exp straight to fp8e4 bits: exp(y) ~= bitcast_fp8(uint8(
# y*8/ln2 + (56 - 0.344))). The f32->uint8 DVE conversion saturates
# negatives to 0 (verified on HW), which flushes negligible-weight keys.
K1P = float(8.0 / np.log(2.0))
K2P = 56.0 - 0.344

# mode: "bf16" | "fp8"; noff: chunks per pass whose exp runs on DVE(+Pool)
MODE = "fp8"
NOFF = 12
V_RESID = False  # extra fp8 residual matmul for V (halves V quantization err)


def build_attention_bass(s=S, sq=SQ, sw=1024, mode=MODE, noff=NOFF,
                         v_resid=V_RESID, n_warm=18):
    """Build the single-core SPMD Bass program."""
    import concourse.mybir as mybir
    import concourse.tile as tile
    from concourse import bacc
    from contextlib import ExitStack

    f32 = mybir.dt.float32
    bf16 = mybir.dt.bfloat16
    fp8 = mybir.dt.float8e4
    u8 = mybir.dt.uint8
    FT = mybir.ActivationFunctionType
    ALU = mybir.AluOpType
    DR = mybir.MatmulPerfMode.DoubleRow

    tch = s // 128          # key chunks
    n_pass = sq // sw       # query passes
    nw = min(512, sw)       # matmul N width (PSUM bank limit for f32 out)
    jn = sw // nw           # matmuls per pass-width
    nk = s // 512           # k'' projection matmuls
    assert tch % 2 == 0 and sq % sw == 0 and sw % nw == 0

    # which chunks' exp is offloaded to DVE(+Pool): last chunk of each of
    # noff equal buckets (uniform spread, skips chunk 0 when noff < tch)
    off_set = {c for c in range(tch)
               if (c * noff) // tch != ((c + 1) * noff) // tch}

    nc = bacc.Bacc("TRN2", target_bir_lowering=False, debug=False)

    xT = nc.dram_tensor("xT", [D, s], bf16, kind="ExternalInput").ap()
    gT = nc.dram_tensor("gT", [D, D], bf16, kind="ExternalInput").ap()
    wvT = nc.dram_tensor("wvT", [D, D + 2], bf16, kind="ExternalInput").ap()
    bv = nc.dram_tensor("bv", [D, 1], f32, kind="ExternalInput").ap()
    out_d = nc.dram_tensor("out", [D, sq], f32, kind="ExternalOutput").ap()

    with tile.TileContext(nc) as tc, ExitStack() as ctx:
        const = ctx.enter_context(tc.tile_pool(name="const", bufs=1))
        big = ctx.enter_context(tc.tile_pool(name="big", bufs=1))
        exp_pool = ctx.enter_context(tc.tile_pool(name="exp", bufs=4))
        epi = ctx.enter_context(tc.tile_pool(name="epi", bufs=2))
        outp = ctx.enter_context(tc.tile_pool(name="outp", bufs=3))
        g_sb = const.tile([D, D], bf16, tag="g")
        wv_sb = const.tile([D, D + 2], bf16, tag="wv")
        bv_sb = const.tile([D, 1], f32, tag="bv")

        xT_sb = big.tile([D, s], bf16, tag="xT")
        # input DMAs: weights first, then x^T in 512-col chunks (cols 0:1024
        # serve both the first k'' chunks and the pass-0 queries)
        nc.sync.dma_start(g_sb[:], gT)
        nc.sync.dma_start(wv_sb[:], wvT)
        for st in range(0, s, 512):
            nc.sync.dma_start(xT_sb[:, st:st + 512], xT[:, st:st + 512])
        nc.sync.dma_start(bv_sb[:], bv)

        # HAM warm-up: dependency-free back-to-back matmuls on zeroed tiles
        # while the DMAs land (~3.4us busy lifts the PE clock 1.2->2.4GHz)
        if n_warm > 0:
            wz = const.tile([128, 512], bf16, tag="wz")
            nc.vector.memset(wz[:], 0.0)
            with tc.tile_pool(name="wups", bufs=1, space="PSUM") as wups:
                wu = wups.tile([128, 512], f32, tag="wu")
                for _ in range(n_warm):
                    nc.tensor.matmul(wu[:], wz[:, :128], wz[:])

        kt_sb = big.tile([D, s], bf16, tag="kt")
        alpha_sb = big.tile([128, tch], f32, tag="alpha")
        if mode == "fp8":
            v8_sb = big.tile([128, 128 * tch], fp8, tag="v8")
            ones8 = const.tile([128, 256], fp8, tag="ones8")
            nc.vector.memset(ones8[:], 1.0)
            if v_resid:
                vr8_sb = big.tile([128, 128 * tch], fp8, tag="vr8")
        else:
            v_sb = big.tile([128, 128 * tch], bf16, tag="v")
            ones_sb = const.tile([128, 128], bf16, tag="ones")
            nc.vector.memset(ones_sb[:], 1.0)
            accdv = ctx.enter_context(tc.tile_pool(name="accdv", bufs=2))
        if noff > 0:
            alpha3_sb = big.tile([128, tch], f32, tag="alpha3")

        with tc.tile_pool(name="scps", bufs=2, space="PSUM") as scps:
            # projection pools live only until the pass loop starts
            qkv_ctx = ExitStack()
            kps = qkv_ctx.enter_context(
                tc.tile_pool(name="kps", bufs=2, space="PSUM"))
            vps = qkv_ctx.enter_context(
                tc.tile_pool(name="vps", bufs=2, space="PSUM"))

            def emit_k(j):
                st = j * 512
                kp = kps.tile([128, 512], f32, tag="kp")
                nc.tensor.matmul(kp[:], g_sb[:], xT_sb[:, st:st + 512])
                nc.vector.tensor_copy(kt_sb[:, st:st + 512], kp[:])

            def emit_v_alpha(c):
                vp = vps.tile([128, D + 2], f32, tag="vp")
                xc = xT_sb[:, c * 128:(c + 1) * 128]
                nc.tensor.matmul(vp[:], xc, wv_sb[:])
                # alpha column with the exp shift folded in
                nc.vector.tensor_scalar_add(
                    alpha_sb[:, c:c + 1], vp[:, D:D + 1], -SHIFT)
                dst = slice(c * 128, (c + 1) * 128)
                if mode == "fp8":
                    if c % 2 == 0:
                        nc.vector.tensor_copy(v8_sb[:, dst], vp[:, :D])
                    else:
                        nc.scalar.activation(v8_sb[:, dst], vp[:, :D],
                                             FT.Copy, bias=0.0, scale=1.0)
                    if v_resid:
                        nc.vector.tensor_tensor(
                            vr8_sb[:, dst], vp[:, :D], v8_sb[:, dst],
                            ALU.subtract)
                else:
                    nc.vector.tensor_copy(v_sb[:, dst], vp[:, :D])

            pair_tiles = {}

            def emit_scores_exp(p, c):
                """scores chunk c of pass p -> PSUM; exp -> et (half-)tile."""
                sc = scps.tile([128, sw], f32, tag="sc")
                kc = kt_sb[:, c * 128:(c + 1) * 128]
                for j in range(jn):
                    nc.tensor.matmul(
                        sc[:, j * nw:(j + 1) * nw], kc,
                        xT_sb[:, p * sw + j * nw: p * sw + (j + 1) * nw])
                abias = alpha_sb[:, c:c + 1]
                if mode == "fp8":
                    if c % 2 == 0:
                        pair_tiles[(p, c // 2)] = exp_pool.tile(
                            [128, 2 * sw], fp8, tag="et", name="et_pair")
                    et = pair_tiles[(p, c // 2)]
                    half = et[:, (c % 2) * sw:(c % 2 + 1) * sw]
                    if c in off_set:
                        # Schraudolph exp straight into the fp8 half-tile
                        nc.vector.tensor_scalar(
                            half.bitcast(u8), sc[:], K1P,
                            alpha3_sb[:, c:c + 1], ALU.mult, ALU.add)
                    else:
                        nc.scalar.activation(half, sc[:], FT.Exp,
                                             bias=abias, scale=1.0)
                    return None
                et = exp_pool.tile([128, sw], bf16, tag="et")
                nc.scalar.activation(et[:], sc[:], FT.Exp,
                                     bias=abias, scale=1.0)
                return et

            def epilogue(p, acc_o, acc_d):
                # normalize + bias, DMA the [e, s] block out; finer blocks on
                # the last pass so tail DMAs start earlier
                bw = 256 if p == n_pass - 1 else nw
                for b0 in range(0, sw, bw):
                    recip = epi.tile([128, bw], f32, tag="recip")
                    nc.vector.reciprocal_approx_fast(
                        recip[:], acc_d[:, b0:b0 + bw])
                    norm = epi.tile([128, bw], f32, tag="norm")
                    nc.vector.tensor_mul(norm[:], acc_o[:, b0:b0 + bw],
                                         recip[:])
                    norm2 = outp.tile([128, bw], f32, tag="norm2")
                    nc.vector.tensor_scalar_add(norm2[:], norm[:], bv_sb[:])
                    c0 = p * sw + b0
                    nc.sync.dma_start(out_d[:, c0:c0 + bw], norm2[:])

            # ---- projections: enough for the pass-0 pipeline first
            nk_pre = min(2, nk)
            nv_pre = min(4, tch)
            for j in range(nk_pre):
                emit_k(j)
            for c in range(nv_pre):
                emit_v_alpha(c)
            if noff > 0:
                # alpha3 = alpha*8/ln2 + K2P (Schraudolph pre-bias, shift
                # already folded into alpha)
                nc.vector.tensor_scalar(
                    alpha3_sb[:, :nv_pre], alpha_sb[:, :nv_pre], K1P, K2P,
                    ALU.mult, ALU.add)

            sched = [(pp, cc) for pp in range(n_pass) for cc in range(tch)]
            npre = min(4, len(sched))  # chunks of score/exp pre-emitted
            pre = []
            for i in range(npre):
                r = emit_scores_exp(*sched[i])
                if mode != "fp8":
                    pre.append(r)
            cursor = [npre]

            # rest of the projections (fills PE while ACT runs first exps)
            for j in range(nk_pre, nk):
                emit_k(j)
            for c in range(nv_pre, tch):
                emit_v_alpha(c)
            if noff > 0 and nv_pre < tch:
                nc.vector.tensor_scalar(
                    alpha3_sb[:, nv_pre:], alpha_sb[:, nv_pre:], K1P, K2P,
                    ALU.mult, ALU.add)

            qkv_ctx.close()
            acc_ctx = ExitStack()
            accps = acc_ctx.enter_context(
                tc.tile_pool(name="accps", bufs=1, space="PSUM"))

            def emit_upto(idx):
                while cursor[0] < len(sched) and cursor[0] <= idx:
                    pp, cc = sched[cursor[0]]
                    cursor[0] += 1
                    r = emit_scores_exp(pp, cc)
                    if mode != "fp8":
                        pre.append(r)

            # ---- attention passes
            if mode == "fp8":
                ones_ap = ones8[:].rearrange("p (two m) -> p two m", two=2)
                n_pair = tch // 2
                for p in range(n_pass):
                    acc_o = accps.tile([128, sw], f32, tag="acco")
                    acc_d = accps.tile([128, sw], f32, tag="accd")
                    for jp in range(n_pair):
                        # keep the score/exp pipeline ~4 chunks ahead
                        emit_upto(p * tch + jp * 2 + 5)
                        et = pair_tiles.pop((p, jp))
                        et3 = et[:].rearrange("p (two n) -> p two n", two=2)
                        v_ap = v8_sb[:, jp * 256:(jp + 1) * 256].rearrange(
                            "p (two m) -> p two m", two=2)
                        first = jp == 0
                        lastd = jp == n_pair - 1
                        lasto = lastd and not v_resid
                        for j in range(jn):
                            rhs = et3[:, :, j * nw:(j + 1) * nw]
                            ob = acc_o[:, j * nw:(j + 1) * nw]
                            nc.tensor.matmul(ob, v_ap, rhs, start=first,
                                             stop=lasto, perf_mode=DR)
                            nc.tensor.matmul(
                                acc_d[:, j * nw:(j + 1) * nw], ones_ap, rhs,
                                start=first, stop=lastd, perf_mode=DR)
                            if v_resid:
                                vr_ap = vr8_sb[
                                    :, jp * 256:(jp + 1) * 256].rearrange(
                                    "p (two m) -> p two m", two=2)
                                nc.tensor.matmul(ob, vr_ap, rhs, start=False,
                                                 stop=lastd, perf_mode=DR)
                    epilogue(p, acc_o, acc_d)
            else:
                for p in range(n_pass):
                    acc_o = accps.tile([128, sw], f32, tag="acco")
                    acc_d = accps.tile([128, sw], f32, tag="accd")
                    acc_dv = accdv.tile([128, sw], bf16, tag="accdv")
                    for c in range(tch):
                        et = pre.pop(0)
                        emit_upto(p * tch + c + 2)
                        vc = v_sb[:, c * 128:(c + 1) * 128]
                        for j in range(jn):
                            nc.tensor.matmul(
                                acc_o[:, j * nw:(j + 1) * nw], vc,
                                et[:, j * nw:(j + 1) * nw],
                                start=(c == 0), stop=(c == tch - 1))
                        if c == 0:
                            nc.vector.tensor_copy(acc_dv[:], et[:])
                        else:
                            nc.vector.tensor_add(acc_dv[:], acc_dv[:], et[:])
                    for j in range(jn):
                        nc.tensor.matmul(acc_d[:, j * nw:(j + 1) * nw],
                                         ones_sb[:],
                                         acc_dv[:, j * nw:(j + 1) * nw],
                                         start=True, stop=True)
                    epilogue(p, acc_o, acc_d)
            acc_ctx.close()
    nc.compile()
    return nc


def make_in_maps(x, Wq, bq, Wk, Wv, bv, s=S, sq=SQ, n_cores=N_CORES):
    """Per-core input dict list. Core c -> batch c//(cores per batch);
    x^T is rolled so the core's query slice sits at columns 0:sq."""
    bf = ml_dtypes.bfloat16
    x = np.asarray(x, np.float64)
    nb = x.shape[0]
    per_b = n_cores // nb
    d = x.shape[2]
    g_t = (np.asarray(Wk, np.float64).T @ np.asarray(Wq, np.float64)
           / np.sqrt(d))
    wtl = (np.asarray(Wk, np.float64).T @ np.asarray(bq, np.float64)
           / np.sqrt(d)).reshape(d, 1)
    wv_t = np.asarray(Wv, np.float64).T
    wv_aug = np.concatenate([wv_t, wtl, wtl], axis=1)
    bvc = np.asarray(bv, np.float32).reshape(d, 1)
    maps = []
    for c in range(n_cores):
        b, h = c // per_b, c % per_b
        xt = x[b].T  # [d, s]
        roll = np.concatenate([xt[:, h * sq:], xt[:, :h * sq]], axis=1)
        maps.append({
            "xT": np.ascontiguousarray(roll.astype(bf)),
            "gT": np.ascontiguousarray(g_t.astype(bf)),
            "wvT": np.ascontiguousarray(wv_aug.astype(bf)),
            "bv": np.ascontiguousarray(bvc),
        })
    return maps


_NC_CACHE = {}


def _get_nc():
    if "nc" not in _NC_CACHE:
        _NC_CACHE["nc"] = build_attention_bass()
    return _NC_CACHE["nc"]


def run_on_hw(inputs, trace=False, **kw):
    from concourse.bass_utils import run_bass_kernel_spmd
    nc = _get_nc()
    maps = make_in_maps(inputs["x"], inputs["Wq"], inputs["bq"], inputs["Wk"],
                        inputs["Wv"], inputs["bv"])
    res = run_bass_kernel_spmd(nc, maps, core_ids=list(range(N_CORES)),
                               trace=trace, **kw)
    nb = np.asarray(inputs["x"]).shape[0]
    per_b = N_CORES // nb
    out = np.empty((nb, S * D), np.float32)
    for c in range(N_CORES):
        b, h = c // per_b, c % per_b
        # device returns out^T [D, SQ]; final layout flip happens here
        out[b, h * SQ * D:(h + 1) * SQ * D] = \
            np.asarray(res.results[c]["out"]).T.reshape(-1)
    return out, res


def kernel(**inputs):
    out, _ = run_on_hw(inputs, trace=False)
    return out


# revision 14
# speedup vs baseline: 1.1853x; 1.1853x over previous
"""Trainium2 Bass kernel for nn_AttentionLayer (B=4, S=4096, D=128, fp32).

Sharding: batch (4) x query-half (2) across 8 NeuronCores. Each core computes
single-head attention for one batch element over a 2048-query slice with full
4096-key context.

Math (host-side algebra):
  scores[s,t] = q[s]@k[t]/sqrt(d) = x[s] G x[t]^T + alpha[t] + const(s),
  G = Wq^T Wk / sqrt(d), alpha[t] = x[t]@(Wk^T bq)/sqrt(d).
  const(s) cancels in softmax; bk drops entirely; bv is added at the end.
  The Q projection disappears: k''[t] = x[t] G^T is the only "key" tensor and
  raw x columns are the scores moving operand. Key order is free (softmax
  sums over keys), so the host rolls x^T per core to put the core's query
  slice at columns 0:SQ - no separate query buffer or DMA.

Per-core dataflow (mode-dependent):
  k''^T = gT.T @ x^T                  (PE, bf16, N=512 x8)
  [V | alpha] chunks = x^T-chunk.T @ wv_aug   (PE, bf16, N=130 x32)
  scoresT[t-chunk, s] = k''-chunk.T @ x^T[:, queries]   (PE bf16 -> PSUM f32)
  et = exp(scores + alpha - SHIFT)    (ACT, PSUM -> SBUF)
  mode "bf16": et bf16; AV bf16 matmuls; denominator chunk-accumulated on DVE
    (bf16 2x) folded by one ones-matmul per pass.
  mode "fp8": et fp8e4 written into pair tiles [128, 2*sw]; AV and denominator
    are fp8 DoubleRow matmuls (2 key-chunks per matmul); denominator costs no
    DVE time. Optionally (noff>0) some chunks' exp runs on DVE via a
    Schraudolph bit-trick straight to fp8 bits (one tensor_scalar per chunk),
    offloading the ACT engine.
  out^T = AV / denom + bv  (DVE), DMA'd out as [e, s]; host flips layout.

A dependency-free warm-up burst of matmuls on zeroed tiles runs while the
input DMAs land: the PE's HAM clock-gate needs ~3.4us of sustained busy to
lift the 1.2GHz cold throttle to 2.4GHz; without the burst the projection
phase runs cold and the gate only lifts ~25us into the kernel.
"""

import sys

import numpy as np

for _p in ("/opt/trn_rl_repo", "/opt/pypackages"):
    if _p not in sys.path:
        sys.path.append(_p)

import ml_dtypes

B, S, D = 4, 4096, 128
N_CORES = 8
SQ = S // 2  # queries per core
SHIFT = 3.5  # subtracted inside exp; cancels in softmax, keeps et fp8-safe

# Schraudolph exp straight to fp8e4 bits: exp(y) ~= bitcast_fp8(uint8(
# y*8/ln2 + (56 - 0.344))). The f32->uint8 DVE conversion saturates
# negatives to 0 (verified on HW), which flushes negligible-weight keys.
K1P = float(8.0 / np.log(2.0))
K2P = 56.0 - 0.344

# mode: "bf16" | "fp8"; noff: chunks per pass whose exp runs on DVE(+Pool)
MODE = "fp8"
NOFF = 12
V_RESID = False  # extra fp8 residual matmul for V (halves V quantization err)


def build_attention_bass(s=S, sq=SQ, sw=512, mode=MODE, noff=NOFF,
                         v_resid=V_RESID, n_warm=18):
    """Build the single-core SPMD Bass program."""
    import concourse.mybir as mybir
    import concourse.tile as tile
    from concourse import bacc
    from contextlib import ExitStack

    f32 = mybir.dt.float32
    bf16 = mybir.dt.bfloat16
    fp8 = mybir.dt.float8e4
    u8 = mybir.dt.uint8
    FT = mybir.ActivationFunctionType
    ALU = mybir.AluOpType
    DR = mybir.MatmulPerfMode.DoubleRow

    tch = s // 128          # key chunks
    n_pass = sq // sw       # query passes
    nw = min(512, sw)       # matmul N width (PSUM bank limit for f32 out)
    jn = sw // nw           # matmuls per pass-width
    nk = s // 512           # k'' projection matmuls
    assert tch % 2 == 0 and sq % sw == 0 and sw % nw == 0

    # which chunks' exp is offloaded to DVE(+Pool): last chunk of each of
    # noff equal buckets (uniform spread, skips chunk 0 when noff < tch)
    off_set = {c for c in range(tch)
               if (c * noff) // tch != ((c + 1) * noff) // tch}

    nc = bacc.Bacc("TRN2", target_bir_lowering=False, debug=False)

    xT = nc.dram_tensor("xT", [D, s], bf16, kind="ExternalInput").ap()
    gT = nc.dram_tensor("gT", [D, D], bf16, kind="ExternalInput").ap()
    wvT = nc.dram_tensor("wvT", [D, D + 2], bf16, kind="ExternalInput").ap()
    bv = nc.dram_tensor("bv", [D, 1], f32, kind="ExternalInput").ap()
    out_d = nc.dram_tensor("out", [D, sq], f32, kind="ExternalOutput").ap()

    with tile.TileContext(nc) as tc, ExitStack() as ctx:
        const = ctx.enter_context(tc.tile_pool(name="const", bufs=1))
        big = ctx.enter_context(tc.tile_pool(name="big", bufs=1))
        exp_pool = ctx.enter_context(tc.tile_pool(name="exp", bufs=6))
        epi = ctx.enter_context(tc.tile_pool(name="epi", bufs=2))
        outp = ctx.enter_context(tc.tile_pool(name="outp", bufs=3))
        g_sb = const.tile([D, D], bf16, tag="g")
        wv_sb = const.tile([D, D + 2], bf16, tag="wv")
        bv_sb = const.tile([D, 1], f32, tag="bv")

        xT_sb = big.tile([D, s], bf16, tag="xT")
        # input DMAs: weights first, then x^T in 512-col chunks (cols 0:1024
        # serve both the first k'' chunks and the pass-0 queries)
        nc.sync.dma_start(g_sb[:], gT)
        nc.sync.dma_start(wv_sb[:], wvT)
        for st in range(0, s, 512):
            nc.sync.dma_start(xT_sb[:, st:st + 512], xT[:, st:st + 512])
        nc.sync.dma_start(bv_sb[:], bv)

        # HAM warm-up: dependency-free back-to-back matmuls on zeroed tiles
        # while the DMAs land (~3.4us busy lifts the PE clock 1.2->2.4GHz)
        if n_warm > 0:
            wz = const.tile([128, 512], bf16, tag="wz")
            nc.vector.memset(wz[:], 0.0)
            with tc.tile_pool(name="wups", bufs=1, space="PSUM") as wups:
                wu = wups.tile([128, 512], f32, tag="wu")
                for _ in range(n_warm):
                    nc.tensor.matmul(wu[:], wz[:, :128], wz[:])

        kt_sb = big.tile([D, s], bf16, tag="kt")
        alpha_sb = big.tile([128, tch], f32, tag="alpha")
        if mode == "fp8":
            v8_sb = big.tile([128, 128 * tch], fp8, tag="v8")
            ones8 = const.tile([128, 256], fp8, tag="ones8")
            nc.vector.memset(ones8[:], 1.0)
            if v_resid:
                vr8_sb = big.tile([128, 128 * tch], fp8, tag="vr8")
        else:
            v_sb = big.tile([128, 128 * tch], bf16, tag="v")
            ones_sb = const.tile([128, 128], bf16, tag="ones")
            nc.vector.memset(ones_sb[:], 1.0)
            accdv = ctx.enter_context(tc.tile_pool(name="accdv", bufs=2))
        if noff > 0:
            alpha3_sb = big.tile([128, tch], f32, tag="alpha3")

        with tc.tile_pool(name="scps", bufs=3, space="PSUM") as scps:
            # projection pools stay open through pass 0: k''/V emission is
            # demand-driven from the score/exp cursor so the copies spread
            # across the ACT/DVE queues instead of clumping ahead of the exps
            qkv_ctx = ExitStack()
            kps = qkv_ctx.enter_context(
                tc.tile_pool(name="kps", bufs=1, space="PSUM"))
            vps = qkv_ctx.enter_context(
                tc.tile_pool(name="vps", bufs=2, space="PSUM"))

            def emit_k(j):
                st = j * 512
                kp = kps.tile([128, 512], f32, tag="kp")
                nc.tensor.matmul(kp[:], g_sb[:], xT_sb[:, st:st + 512])
                nc.vector.tensor_copy(kt_sb[:, st:st + 512], kp[:])

            def emit_v_alpha(c):
                vp = vps.tile([128, D + 2], f32, tag="vp")
                xc = xT_sb[:, c * 128:(c + 1) * 128]
                nc.tensor.matmul(vp[:], xc, wv_sb[:])
                # alpha column with the exp shift folded in
                nc.vector.tensor_scalar_add(
                    alpha_sb[:, c:c + 1], vp[:, D:D + 1], -SHIFT)
                if noff > 0 and c in off_set:
                    # Schraudolph pre-bias for this chunk
                    nc.vector.tensor_scalar(
                        alpha3_sb[:, c:c + 1], alpha_sb[:, c:c + 1],
                        K1P, K2P, ALU.mult, ALU.add)
                dst = slice(c * 128, (c + 1) * 128)
                if mode == "fp8":
                    if c % 2 == 0:
                        nc.vector.tensor_copy(v8_sb[:, dst], vp[:, :D])
                    else:
                        nc.scalar.activation(v8_sb[:, dst], vp[:, :D],
                                             FT.Copy, bias=0.0, scale=1.0)
                    if v_resid:
                        nc.vector.tensor_tensor(
                            vr8_sb[:, dst], vp[:, :D], v8_sb[:, dst],
                            ALU.subtract)
                else:
                    nc.vector.tensor_copy(v_sb[:, dst], vp[:, :D])

            pair_tiles = {}

            def emit_scores_exp(p, c):
                """scores chunk c of pass p -> PSUM; exp -> et (half-)tile."""
                sc = scps.tile([128, sw], f32, tag="sc")
                kc = kt_sb[:, c * 128:(c + 1) * 128]
                for j in range(jn):
                    nc.tensor.matmul(
                        sc[:, j * nw:(j + 1) * nw], kc,
                        xT_sb[:, p * sw + j * nw: p * sw + (j + 1) * nw])
                abias = alpha_sb[:, c:c + 1]
                if mode == "fp8":
                    if c % 2 == 0:
                        pair_tiles[(p, c // 2)] = exp_pool.tile(
                            [128, 2 * sw], fp8, tag="et", name="et_pair")
                    et = pair_tiles[(p, c // 2)]
                    half = et[:, (c % 2) * sw:(c % 2 + 1) * sw]
                    if c in off_set:
                        # Schraudolph exp straight into the fp8 half-tile
                        nc.vector.tensor_scalar(
                            half.bitcast(u8), sc[:], K1P,
                            alpha3_sb[:, c:c + 1], ALU.mult, ALU.add)
                    else:
                        nc.scalar.activation(half, sc[:], FT.Exp,
                                             bias=abias, scale=1.0)
                    return None
                et = exp_pool.tile([128, sw], bf16, tag="et")
                nc.scalar.activation(et[:], sc[:], FT.Exp,
                                     bias=abias, scale=1.0)
                return et

            def epilogue(p, acc_o, acc_d):
                # normalize + bias, DMA the [e, s] block out; finer blocks on
                # the last pass so tail DMAs start earlier
                bw = 256 if p == n_pass - 1 else nw
                for b0 in range(0, sw, bw):
                    recip = epi.tile([128, bw], f32, tag="recip")
                    nc.vector.reciprocal_approx_fast(
                        recip[:], acc_d[:, b0:b0 + bw])
                    norm = epi.tile([128, bw], f32, tag="norm")
                    nc.vector.tensor_mul(norm[:], acc_o[:, b0:b0 + bw],
                                         recip[:])
                    norm2 = outp.tile([128, bw], f32, tag="norm2")
                    nc.vector.tensor_scalar_add(norm2[:], norm[:], bv_sb[:])
                    c0 = p * sw + b0
                    nc.sync.dma_start(out_d[:, c0:c0 + bw], norm2[:])

            # lazily-advancing projection cursors (k-mm j covers score
            # chunks 4j..4j+3; V chunk c feeds AV pair c//2)
            k_next = [0]
            v_next = [0]

            def ensure_proj(cc):
                while k_next[0] < min(nk, cc // 4 + 2):
                    emit_k(k_next[0])
                    k_next[0] += 1
                while v_next[0] < min(tch, cc + 4):
                    emit_v_alpha(v_next[0])
                    v_next[0] += 1

            sched = [(pp, cc) for pp in range(n_pass) for cc in range(tch)]
            npre = min(6, len(sched))  # chunks of score/exp pre-emitted
            pre = []
            cursor = [0]

            def emit_upto(idx):
                while cursor[0] < len(sched) and cursor[0] <= idx:
                    pp, cc = sched[cursor[0]]
                    cursor[0] += 1
                    ensure_proj(cc if pp == 0 else tch)
                    r = emit_scores_exp(pp, cc)
                    if mode != "fp8":
                        pre.append(r)

            emit_upto(npre - 1)
            acc_ctx = ExitStack()
            accps = acc_ctx.enter_context(
                tc.tile_pool(name="accps", bufs=1, space="PSUM"))

            # ---- attention passes
            if mode == "fp8":
                ones_ap = ones8[:].rearrange("p (two m) -> p two m", two=2)
                n_pair = tch // 2
                for p in range(n_pass):
                    acc_o = accps.tile([128, sw], f32, tag="acco")
                    acc_d = accps.tile([128, sw], f32, tag="accd")
                    for jp in range(n_pair):
                        # keep the score/exp pipeline ~6 chunks ahead
                        emit_upto(p * tch + jp * 2 + 7)
                        et = pair_tiles.pop((p, jp))
                        et3 = et[:].rearrange("p (two n) -> p two n", two=2)
                        v_ap = v8_sb[:, jp * 256:(jp + 1) * 256].rearrange(
                            "p (two m) -> p two m", two=2)
                        first = jp == 0
                        lastd = jp == n_pair - 1
                        lasto = lastd and not v_resid
                        for j in range(jn):
                            rhs = et3[:, :, j * nw:(j + 1) * nw]
                            ob = acc_o[:, j * nw:(j + 1) * nw]
                            nc.tensor.matmul(ob, v_ap, rhs, start=first,
                                             stop=lasto, perf_mode=DR)
                            nc.tensor.matmul(
                                acc_d[:, j * nw:(j + 1) * nw], ones_ap, rhs,
                                start=first, stop=lastd, perf_mode=DR)
                            if v_resid:
                                vr_ap = vr8_sb[
                                    :, jp * 256:(jp + 1) * 256].rearrange(
                                    "p (two m) -> p two m", two=2)
                                nc.tensor.matmul(ob, vr_ap, rhs, start=False,
                                                 stop=lastd, perf_mode=DR)
                    epilogue(p, acc_o, acc_d)
            else:
                for p in range(n_pass):
                    acc_o = accps.tile([128, sw], f32, tag="acco")
                    acc_d = accps.tile([128, sw], f32, tag="accd")
                    acc_dv = accdv.tile([128, sw], bf16, tag="accdv")
                    for c in range(tch):
                        et = pre.pop(0)
                        emit_upto(p * tch + c + 2)
                        vc = v_sb[:, c * 128:(c + 1) * 128]
                        for j in range(jn):
                            nc.tensor.matmul(
                                acc_o[:, j * nw:(j + 1) * nw], vc,
                                et[:, j * nw:(j + 1) * nw],
                                start=(c == 0), stop=(c == tch - 1))
                        if c == 0:
                            nc.vector.tensor_copy(acc_dv[:], et[:])
                        else:
                            nc.vector.tensor_add(acc_dv[:], acc_dv[:], et[:])
                    for j in range(jn):
                        nc.tensor.matmul(acc_d[:, j * nw:(j + 1) * nw],
                                         ones_sb[:],
                                         acc_dv[:, j * nw:(j + 1) * nw],
                                         start=True, stop=True)
                    epilogue(p, acc_o, acc_d)
            acc_ctx.close()
            qkv_ctx.close()
    nc.compile()
    return nc


def make_in_maps(x, Wq, bq, Wk, Wv, bv, s=S, sq=SQ, n_cores=N_CORES):
    """Per-core input dict list. Core c -> batch c//(cores per batch);
    x^T is rolled so the core's query slice sits at columns 0:sq."""
    bf = ml_dtypes.bfloat16
    x = np.asarray(x, np.float64)
    nb = x.shape[0]
    per_b = n_cores // nb
    d = x.shape[2]
    g_t = (np.asarray(Wk, np.float64).T @ np.asarray(Wq, np.float64)
           / np.sqrt(d))
    wtl = (np.asarray(Wk, np.float64).T @ np.asarray(bq, np.float64)
           / np.sqrt(d)).reshape(d, 1)
    wv_t = np.asarray(Wv, np.float64).T
    wv_aug = np.concatenate([wv_t, wtl, wtl], axis=1)
    bvc = np.asarray(bv, np.float32).reshape(d, 1)
    maps = []
    for c in range(n_cores):
        b, h = c // per_b, c % per_b
        xt = x[b].T  # [d, s]
        roll = np.concatenate([xt[:, h * sq:], xt[:, :h * sq]], axis=1)
        maps.append({
            "xT": np.ascontiguousarray(roll.astype(bf)),
            "gT": np.ascontiguousarray(g_t.astype(bf)),
            "wvT": np.ascontiguousarray(wv_aug.astype(bf)),
            "bv": np.ascontiguousarray(bvc),
        })
    return maps


_NC_CACHE = {}


def _get_nc():
    if "nc" not in _NC_CACHE:
        _NC_CACHE["nc"] = build_attention_bass()
    return _NC_CACHE["nc"]


def run_on_hw(inputs, trace=False, **kw):
    from concourse.bass_utils import run_bass_kernel_spmd
    nc = _get_nc()
    maps = make_in_maps(inputs["x"], inputs["Wq"], inputs["bq"], inputs["Wk"],
                        inputs["Wv"], inputs["bv"])
    res = run_bass_kernel_spmd(nc, maps, core_ids=list(range(N_CORES)),
                               trace=trace, **kw)
    nb = np.asarray(inputs["x"]).shape[0]
    per_b = N_CORES // nb
    out = np.empty((nb, S * D), np.float32)
    for c in range(N_CORES):
        b, h = c // per_b, c % per_b
        # device returns out^T [D, SQ]; final layout flip happens here
        out[b, h * SQ * D:(h + 1) * SQ * D] = \
            np.asarray(res.results[c]["out"]).T.reshape(-1)
    return out, res


def kernel(**inputs):
    out, _ = run_on_hw(inputs, trace=False)
    return out
